# revision 2
# baseline (speedup 1.0000x reference)
"""Branched feed-forward (4-phase MoE-style FF) on 8 Trainium2 NeuronCores.

Reference computation (B=32, S=1024, D=1024, P=4, F=4096):
    xs = x.reshape(B, P, S//P, D)              # static contiguous phase split
    h  = relu(xs @ W1[p] + b1[p])              # per-phase FF, D -> F
    y  = h @ W2[p] + b2[p]                     # F -> D
    out = y.reshape(B, S, D)

Sharding: 8 cores = 4 phases x 2 F-halves (expert parallel + FF-width
parallel).  Core c handles phase p = c//2, F-half fh = c%2: it computes a
partial y (contraction over its half of F) for ALL 8192 tokens of its
phase.  Host sums the two partials per phase and adds b2 (cheap numpy).

Per-core kernel (all weights SBUF-resident):
    for each token block (TT tokens):
        FF1: h[ft, :] = relu( sum_dc W1[dc,ft].T @ xT[dc, :] + b1[ft] )
        FF2: y[dt, :] = sum_fc W2[fc,dt].T @ h[fc, :]

Matmul dtype MM_DT: "bfloat16" (shipped default; rel err 3.2e-3 vs the
2e-2 gate, ~1.06 ms sustained on-device) or "float32r" (full-rate
single-pass fp32, ~2e-4 rel err, ~1.2 ms).  The kernel is tensor-bound:
NTFF traces show the PE >99% busy at the sustained clock (2.0 GHz; short
bursts boost to 2.4), MM spacing ~N/2.0+1.3 ns, so bf16 @ TT=256 sits at
the sustained-clock roofline.  fp8 DoubleRow (1.44x) is numerically out
of reach (~5-7%% rel err) and bass exposes no int8 matmul.

For timing builds (loop_reps>1), staggered_reset=True on the For_i loop
removes the per-iteration all-engine barrier (worth ~20 us/iteration);
results were verified identical.
"""

import numpy as np

import concourse.bacc as bacc
import concourse.mybir as mybir
import concourse.tile as tile
from concourse.bass import ts

# Problem dims (hardcoded per contest contract)
B, S, D = 32, 1024, 1024
P, F = 4, 4096
N_CORES = 8

# Per-core dims
FH = F // 2          # F half per core = 2048
T = B * (S // P)     # tokens per phase = 8192
DC = D // 128        # 8 contraction chunks for FF1 / out tiles for FF2
FT = FH // 128       # 16 out tiles for FF1 / contraction chunks for FF2

# Tunables (defaults = the graded configuration)
MM_DT = "bfloat16"   # matmul dtype: "float32r" | "bfloat16"
TT = 512             # token block (matmul moving free dim)

F32 = mybir.dt.float32


def build_bass(reps=1, loop_reps=1, mm_dt=None, tt=None, staggered_reset=False):
    """Build the per-core Bass program.

    `reps` repeats the compute sweep by instruction duplication; `loop_reps`
    repeats it via a hardware For_i loop (no code growth).  Both are timing
    aids for test.py (slope between rep counts isolates on-device time);
    the graded kernel uses reps=1, loop_reps=1."""
    mm_dt = MM_DT if mm_dt is None else mm_dt
    tt = TT if tt is None else tt
    DT = getattr(mybir.dt, mm_dt)
    tb_n = T // tt

    # SBUF budget (~207.8 KB/partition usable): weights + h + x + y tiles
    esz = mybir.dt.size(DT)
    w_bytes = (DC * FH + FT * D) * esz
    h_bytes = FT * tt * esz
    x_bytes = DC * tt * esz
    y_bytes = tt * 4
    budget = 204 * 1024
    h_bufs = 3
    x_bufs = 8
    while w_bytes + h_bufs * h_bytes + x_bufs * x_bytes + 4 * y_bytes + 256 > budget:
        if x_bufs > 2:
            x_bufs -= 1
        elif h_bufs > 1:
            h_bufs -= 1
        else:
            break

    nc = bacc.Bacc(None, target_bir_lowering=False)

    # Host pre-permutes everything so every DMA line is one contiguous
    # per-partition chunk (x: DC*tt*esz, w1/w2: 64KB, y: tt*4B).
    x_d = nc.dram_tensor("x", [tb_n, 128, DC, tt], DT, kind="ExternalInput")
    w1_d = nc.dram_tensor("w1", [128, DC, FH], DT, kind="ExternalInput")
    w2_d = nc.dram_tensor("w2", [128, FT, D], DT, kind="ExternalInput")
    b1_d = nc.dram_tensor("b1", [128, FT], F32, kind="ExternalInput")
    y_d = nc.dram_tensor("y", [tb_n, DC, 128, tt], F32, kind="ExternalOutput")

    with tile.TileContext(nc) as tc:
        with (
            tc.tile_pool(name="weights", bufs=1) as wpool,
            tc.tile_pool(name="xin", bufs=x_bufs) as xpool,
            tc.tile_pool(name="hbuf", bufs=h_bufs) as hpool,
            tc.tile_pool(name="yout", bufs=4) as ypool,
            tc.tile_pool(name="psum", bufs=8, space="PSUM") as psum,
        ):
            w1_s = wpool.tile([128, DC, FH], DT)
            nc.sync.dma_start(w1_s[:], w1_d[:])
            w2_s = wpool.tile([128, FT, D], DT)
            nc.sync.dma_start(w2_s[:], w2_d[:])
            b1_s = wpool.tile([128, FT], F32)
            nc.sync.dma_start(b1_s[:], b1_d[:])

            def sweep():
                for tb in [t for _ in range(reps) for t in range(tb_n)]:
                    x_t = xpool.tile([128, DC, tt], DT, tag="x")
                    nc.sync.dma_start(x_t[:], x_d[tb])

                    h_t = hpool.tile([128, FT, tt], DT, tag="h")
                    for ft in range(FT):
                        ps = psum.tile([128, tt], F32, tag="ps")
                        for dc in range(DC):
                            nc.tensor.matmul(
                                ps[:],
                                w1_s[:, dc, ts(ft, 128)],
                                x_t[:, dc, :],
                                start=(dc == 0),
                                stop=(dc == DC - 1),
                            )
                        nc.scalar.activation(
                            h_t[:, ft, :],
                            ps[:],
                            mybir.ActivationFunctionType.Relu,
                            bias=b1_s[:, ft : ft + 1],
                        )

                    for dt_ in range(DC):
                        ps = psum.tile([128, tt], F32, tag="ps")
                        for fc in range(FT):
                            nc.tensor.matmul(
                                ps[:],
                                w2_s[:, fc, ts(dt_, 128)],
                                h_t[:, fc, :],
                                start=(fc == 0),
                                stop=(fc == FT - 1),
                            )
                        y_t = ypool.tile([128, tt], F32, tag="y")
                        nc.vector.tensor_copy(y_t[:], ps[:])
                        nc.sync.dma_start(y_d[tb, dt_], y_t[:])

            if loop_reps > 1:
                with tc.For_i(0, loop_reps, 1, staggered_reset=staggered_reset):
                    sweep()
            else:
                sweep()

    nc.compile()
    return nc


def _np_dt(mm_dt=None):
    return mybir.dt.np(getattr(mybir.dt, MM_DT if mm_dt is None else mm_dt))


def _shard_inputs(x, W1, b1, W2, mm_dt=None, tt=None):
    """Build the 8 per-core input maps. Core c: phase c//2, F-half c%2."""
    tt = TT if tt is None else tt
    tb_n = T // tt
    np_dt = _np_dt(mm_dt)
    in_maps = []
    xt_by_phase = {}
    for c in range(N_CORES):
        p, fh = divmod(c, 2)
        if p not in xt_by_phase:  # both F-half cores of a phase share x
            xs = np.ascontiguousarray(x.reshape(B, P, S // P, D)[:, p])
            xt_by_phase[p] = np.ascontiguousarray(
                xs.reshape(tb_n, tt, DC, 128).transpose(0, 3, 2, 1)  # [tbn,128,DC,tt]
            ).astype(np_dt)
        xt = xt_by_phase[p]
        w1 = (
            W1[p][:, fh * FH : (fh + 1) * FH].reshape(DC, 128, FH).transpose(1, 0, 2)
        )
        w2 = (
            W2[p][fh * FH : (fh + 1) * FH, :].reshape(FT, 128, D).transpose(1, 0, 2)
        )
        b1c = b1[p][fh * FH : (fh + 1) * FH].reshape(FT, 128).T
        in_maps.append(
            {
                "x": xt,
                "w1": np.ascontiguousarray(w1).astype(np_dt),
                "w2": np.ascontiguousarray(w2).astype(np_dt),
                "b1": np.ascontiguousarray(b1c).astype(np.float32),
            }
        )
    return in_maps


def _unshard_outputs(results, b2, tt=None):
    """results: list of 8 dicts with 'y' [tb_n,DC,128,tt] partial sums."""
    tt = TT if tt is None else tt
    y = np.empty((B, P, S // P, D), dtype=np.float32)
    for p in range(P):
        ya = results[2 * p]["y"]
        yb = results[2 * p + 1]["y"]
        # [tbn,DC,128,tt] -> [tbn,tt,DC,128] -> [T, D]
        yp = (ya + yb).transpose(0, 3, 1, 2).reshape(T, D) + b2[p][None, :]
        y[:, p] = yp.reshape(B, S // P, D)
    return y.reshape(B, S, D)


# ---------------------------------------------------------------------------
# Compile-once PJRT runner (mirrors concourse.bass2jax.run_bass_via_pjrt but
# caches the sharded executable so repeat kernel() calls skip re-tracing).

_RUNNER = None


def _make_runner():
    import jax
    from jax.sharding import Mesh, PartitionSpec
    from jax.experimental.shard_map import shard_map
    from concourse.bass2jax import (
        _bass_exec_p,
        install_neuronx_cc_hook,
        partition_id_tensor,
    )

    nc = build_bass()
    install_neuronx_cc_hook()

    partition_name = nc.partition_id_tensor.name if nc.partition_id_tensor else None

    in_names, out_names, out_avals = [], [], []
    for alloc in nc.m.functions[0].allocations:
        if not isinstance(alloc, mybir.MemoryLocationSet):
            continue
        name = alloc.memorylocations[0].name
        if alloc.kind == "ExternalInput":
            if name != partition_name:
                in_names.append(name)
        elif alloc.kind == "ExternalOutput":
            out_names.append(name)
            out_avals.append(
                jax.core.ShapedArray(
                    tuple(alloc.tensor_shape), mybir.dt.np(alloc.dtype)
                )
            )
    n_params = len(in_names)
    all_in_names = list(in_names) + list(out_names)
    if partition_name is not None:
        all_in_names.append(partition_name)

    def _body(*args):
        operands = list(args)
        if partition_name is not None:
            operands.append(partition_id_tensor())
        outs = _bass_exec_p.bind(
            *operands,
            out_avals=tuple(out_avals),
            in_names=tuple(all_in_names),
            out_names=tuple(out_names),
            lowering_input_output_aliases=(),
            sim_require_finite=True,
            sim_require_nnan=True,
            nc=nc,
        )
        return tuple(outs)

    devices = jax.devices()[:N_CORES]
    mesh = Mesh(np.asarray(devices), ("core",))
    n_outs = len(out_names)
    jitted = jax.jit(
        shard_map(
            _body,
            mesh=mesh,
            in_specs=(PartitionSpec("core"),) * (n_params + n_outs),
            out_specs=(PartitionSpec("core"),) * n_outs,
            check_rep=False,
        ),
        keep_unused=True,
    )

    def run(in_maps):
        concat_in = [
            np.concatenate(
                [np.asarray(in_maps[c][nm]) for c in range(N_CORES)], axis=0
            )
            for nm in in_names
        ]
        concat_zeros = [
            np.zeros((N_CORES * a.shape[0], *a.shape[1:]), a.dtype)
            for a in out_avals
        ]
        outs = jitted(*concat_in, *concat_zeros)
        return [
            {
                nm: np.asarray(outs[i]).reshape(N_CORES, *out_avals[i].shape)[c]
                for i, nm in enumerate(out_names)
            }
            for c in range(N_CORES)
        ]

    return run


def kernel(x, W1, b1, W2, b2, phases):
    """Full-input entry point. `phases` is unused: the reference's phase
    assignment is the static contiguous partition of the sequence."""
    global _RUNNER
    x = np.asarray(x, dtype=np.float32)
    W1 = np.asarray(W1, dtype=np.float32)
    b1 = np.asarray(b1, dtype=np.float32)
    W2 = np.asarray(W2, dtype=np.float32)
    b2 = np.asarray(b2, dtype=np.float32)

    if _RUNNER is None:
        _RUNNER = _make_runner()
    in_maps = _shard_inputs(x, W1, b1, W2)
    try:
        results = _RUNNER(in_maps)
    except Exception:
        # transient NRT device errors have been observed; retry once
        results = _RUNNER(in_maps)
    return _unshard_outputs(results, b2)


if __name__ == "__main__":
    rng = np.random.default_rng(0)
    x = rng.standard_normal((B, S, D), dtype=np.float32)
    W1 = (rng.random((P, D, F), dtype=np.float32) - 0.5) / np.sqrt(D)
    b1 = (rng.random((P, F), dtype=np.float32) - 0.5) / np.sqrt(D)
    W2 = (rng.random((P, F, D), dtype=np.float32) - 0.5) / np.sqrt(F)
    b2 = (rng.random((P, D), dtype=np.float32) - 0.5) / np.sqrt(F)
    phases = rng.integers(0, P, size=(B, S)).astype(np.int32)

    y = kernel(x, W1, b1, W2, b2, phases)

    xs = x.reshape(B, P, S // P, D)
    h = np.maximum(np.einsum("bpsd,pdf->bpsf", xs, W1) + b1[None, :, None, :], 0.0)
    yref = (np.einsum("bpsf,pfd->bpsd", h, W2) + b2[None, :, None, :]).reshape(B, S, D)
    err = np.linalg.norm(y - yref) / np.linalg.norm(yref)
    print("rel err:", err)



# revision 11
# speedup vs baseline: 1.0083x; 1.0083x over previous
"""Branched feed-forward (4-phase MoE-style FF) on 8 Trainium2 NeuronCores.

Reference computation (B=32, S=1024, D=1024, P=4, F=4096):
    xs = x.reshape(B, P, S//P, D)              # static contiguous phase split
    h  = relu(xs @ W1[p] + b1[p])              # per-phase FF, D -> F
    y  = h @ W2[p] + b2[p]                     # F -> D
    out = y.reshape(B, S, D)

Sharding: 8 cores = 4 phases x 2 F-halves (expert parallel + FF-width
parallel).  Core c handles phase p = c//2, F-half fh = c%2: it computes a
partial y (contraction over its half of F) for ALL 8192 tokens of its
phase.  Host sums the two partials per phase and adds b2 (cheap numpy).

Per-core kernel (all weights SBUF-resident):
    for each token block (TT tokens):
        FF1: h[ft, :] = relu( sum_dc W1[dc,ft].T @ xT[dc, :] + b1[ft] )
        FF2: y[dt, :] = sum_fc W2[fc,dt].T @ h[fc, :]

Matmul dtype MM_DT: "bfloat16" (shipped default; rel err 3.2e-3 vs the
2e-2 gate, ~1.06 ms sustained on-device) or "float32r" (full-rate
single-pass fp32, ~2e-4 rel err, ~1.2 ms).  The kernel is tensor-bound:
NTFF traces show the PE >99% busy at the sustained clock (2.0 GHz; short
bursts boost to 2.4), MM spacing ~N/2.0+1.3 ns, so bf16 @ TT=256 sits at
the sustained-clock roofline.  fp8 DoubleRow (1.44x) is numerically out
of reach (~5-7%% rel err) and bass exposes no int8 matmul.

For timing builds (loop_reps>1), staggered_reset=True on the For_i loop
removes the per-iteration all-engine barrier (worth ~20 us/iteration);
results were verified identical.
"""

import numpy as np

import concourse.bacc as bacc
import concourse.mybir as mybir
import concourse.tile as tile
from concourse.bass import ts

# Problem dims (hardcoded per contest contract)
B, S, D = 32, 1024, 1024
P, F = 4, 4096
N_CORES = 8

# Per-core dims
FH = F // 2          # F half per core = 2048
T = B * (S // P)     # tokens per phase = 8192
DC = D // 128        # 8 contraction chunks for FF1 / out tiles for FF2
FT = FH // 128       # 16 out tiles for FF1 / contraction chunks for FF2

# Tunables (defaults = the graded configuration)
MM_DT = "bfloat16"   # matmul dtype: "float32r" | "bfloat16"
TT = 512             # token block (matmul moving free dim)
Y_BF16 = True        # emit y partials in bf16 (halves y DMA traffic)

F32 = mybir.dt.float32


def build_bass(reps=1, loop_reps=1, mm_dt=None, tt=None, staggered_reset=False):
    """Build the per-core Bass program.

    `reps` repeats the compute sweep by instruction duplication; `loop_reps`
    repeats it via a hardware For_i loop (no code growth).  Both are timing
    aids for test.py (slope between rep counts isolates on-device time);
    the graded kernel uses reps=1, loop_reps=1."""
    mm_dt = MM_DT if mm_dt is None else mm_dt
    tt = TT if tt is None else tt
    DT = getattr(mybir.dt, mm_dt)
    tb_n = T // tt

    Y_DT = mybir.dt.bfloat16 if Y_BF16 else F32

    # SBUF budget (~207.8 KB/partition usable): weights + h + x + y tiles
    esz = mybir.dt.size(DT)
    w_bytes = (DC * FH + FT * D) * esz
    h_bytes = FT * tt * esz
    x_bytes = DC * tt * esz
    y_bytes = tt * mybir.dt.size(Y_DT)
    budget = 204 * 1024
    h_bufs = 3
    x_bufs = 8
    while w_bytes + h_bufs * h_bytes + x_bufs * x_bytes + 4 * y_bytes + 256 > budget:
        if x_bufs > 2:
            x_bufs -= 1
        elif h_bufs > 1:
            h_bufs -= 1
        else:
            break

    nc = bacc.Bacc(None, target_bir_lowering=False)

    # Host pre-permutes everything so every DMA line is one contiguous
    # per-partition chunk (x: DC*tt*esz, w1/w2: 64KB, y: tt*4B).
    x_d = nc.dram_tensor("x", [tb_n, 128, DC, tt], DT, kind="ExternalInput")
    w1_d = nc.dram_tensor("w1", [128, DC, FH], DT, kind="ExternalInput")
    w2_d = nc.dram_tensor("w2", [128, FT, D], DT, kind="ExternalInput")
    b1_d = nc.dram_tensor("b1", [128, FT], F32, kind="ExternalInput")
    y_d = nc.dram_tensor("y", [tb_n, DC, 128, tt], Y_DT, kind="ExternalOutput")

    # Weights arrive as 4 chunked tiles each so the first FF1/FF2 matmuls can
    # start after ~1/4 of the weight DMA instead of all of it (single-sweep
    # startup; steady-state/loop timing is unaffected — weights load once).
    NQ = 4
    w1c = FH // NQ   # 512 cols of FH per chunk -> ft range [4q, 4q+4)
    w2c = D // NQ    # 256 cols of D per chunk  -> dt range [2q, 2q+2)

    with tile.TileContext(nc) as tc:
        with (
            tc.tile_pool(name="weights", bufs=1) as wpool,
            tc.tile_pool(name="xin", bufs=x_bufs) as xpool,
            tc.tile_pool(name="hbuf", bufs=h_bufs) as hpool,
            tc.tile_pool(name="yout", bufs=4) as ypool,
            tc.tile_pool(name="psum", bufs=8, space="PSUM") as psum,
        ):
            w1_qs, w2_qs = [], []
            for q in range(NQ):
                t = wpool.tile([128, DC, w1c], DT)
                nc.sync.dma_start(t[:], w1_d[:, :, q * w1c : (q + 1) * w1c])
                w1_qs.append(t)
            for q in range(NQ):
                t = wpool.tile([128, FT, w2c], DT)
                nc.sync.dma_start(t[:], w2_d[:, :, q * w2c : (q + 1) * w2c])
                w2_qs.append(t)
            b1_s = wpool.tile([128, FT], F32)
            nc.sync.dma_start(b1_s[:], b1_d[:])

            # PE warmup: ~4us of throwaway matmuls on zeroed scratch while the
            # weight DMAs stream, so the HAM clock gate reaches 8/8 (2.4 GHz)
            # before the first real matmul (single-sweep startup only).
            wz = wpool.tile([128, 128], DT)
            nc.vector.memset(wz[:], 0)
            xz = wpool.tile([128, tt], DT)
            nc.vector.memset(xz[:], 0)
            pz = psum.tile([128, tt], F32, tag="ps")
            n_warm = max(2, 5120 // tt)  # ~4.3us at the 1.2 GHz cold clock
            for i in range(n_warm):
                nc.tensor.matmul(
                    pz[:], wz[:], xz[:], start=(i == 0), stop=(i == n_warm - 1)
                )
            nc.vector.tensor_copy(wz[:, 0:1], pz[:, 0:1])

            def sweep():
                for tb in [t for _ in range(reps) for t in range(tb_n)]:
                    x_t = xpool.tile([128, DC, tt], DT, tag="x")
                    nc.sync.dma_start(x_t[:], x_d[tb])

                    h_t = hpool.tile([128, FT, tt], DT, tag="h")
                    for ft in range(FT):
                        ps = psum.tile([128, tt], F32, tag="ps")
                        w1_q = w1_qs[ft // (FT // NQ)]
                        for dc in range(DC):
                            nc.tensor.matmul(
                                ps[:],
                                w1_q[:, dc, ts(ft % (FT // NQ), 128)],
                                x_t[:, dc, :],
                                start=(dc == 0),
                                stop=(dc == DC - 1),
                            )
                        nc.scalar.activation(
                            h_t[:, ft, :],
                            ps[:],
                            mybir.ActivationFunctionType.Relu,
                            bias=b1_s[:, ft : ft + 1],
                        )

                    for dt_ in range(DC):
                        ps = psum.tile([128, tt], F32, tag="ps")
                        w2_q = w2_qs[dt_ // (DC // NQ)]
                        for fc in range(FT):
                            nc.tensor.matmul(
                                ps[:],
                                w2_q[:, fc, ts(dt_ % (DC // NQ), 128)],
                                h_t[:, fc, :],
                                start=(fc == 0),
                                stop=(fc == FT - 1),
                            )
                        y_t = ypool.tile([128, tt], Y_DT, tag="y")
                        nc.vector.tensor_copy(y_t[:], ps[:])
                        nc.sync.dma_start(y_d[tb, dt_], y_t[:])

            if loop_reps > 1:
                with tc.For_i(0, loop_reps, 1, staggered_reset=staggered_reset):
                    sweep()
            else:
                sweep()

    nc.compile()
    return nc


def _np_dt(mm_dt=None):
    return mybir.dt.np(getattr(mybir.dt, MM_DT if mm_dt is None else mm_dt))


def _shard_inputs(x, W1, b1, W2, mm_dt=None, tt=None):
    """Build the 8 per-core input maps. Core c: phase c//2, F-half c%2."""
    tt = TT if tt is None else tt
    tb_n = T // tt
    np_dt = _np_dt(mm_dt)
    in_maps = []
    xt_by_phase = {}
    for c in range(N_CORES):
        p, fh = divmod(c, 2)
        if p not in xt_by_phase:  # both F-half cores of a phase share x
            xs = np.ascontiguousarray(x.reshape(B, P, S // P, D)[:, p])
            xt_by_phase[p] = np.ascontiguousarray(
                xs.reshape(tb_n, tt, DC, 128).transpose(0, 3, 2, 1)  # [tbn,128,DC,tt]
            ).astype(np_dt)
        xt = xt_by_phase[p]
        w1 = (
            W1[p][:, fh * FH : (fh + 1) * FH].reshape(DC, 128, FH).transpose(1, 0, 2)
        )
        w2 = (
            W2[p][fh * FH : (fh + 1) * FH, :].reshape(FT, 128, D).transpose(1, 0, 2)
        )
        b1c = b1[p][fh * FH : (fh + 1) * FH].reshape(FT, 128).T
        in_maps.append(
            {
                "x": xt,
                "w1": np.ascontiguousarray(w1).astype(np_dt),
                "w2": np.ascontiguousarray(w2).astype(np_dt),
                "b1": np.ascontiguousarray(b1c).astype(np.float32),
            }
        )
    return in_maps


def _unshard_outputs(results, b2, tt=None):
    """results: list of 8 dicts with 'y' [tb_n,DC,128,tt] partial sums."""
    tt = TT if tt is None else tt
    y = np.empty((B, P, S // P, D), dtype=np.float32)
    for p in range(P):
        ya = results[2 * p]["y"].astype(np.float32)
        yb = results[2 * p + 1]["y"].astype(np.float32)
        # [tbn,DC,128,tt] -> [tbn,tt,DC,128] -> [T, D]
        yp = (ya + yb).transpose(0, 3, 1, 2).reshape(T, D) + b2[p][None, :]
        y[:, p] = yp.reshape(B, S // P, D)
    return y.reshape(B, S, D)


# ---------------------------------------------------------------------------
# Compile-once PJRT runner (mirrors concourse.bass2jax.run_bass_via_pjrt but
# caches the sharded executable so repeat kernel() calls skip re-tracing).

_RUNNER = None


def _make_runner():
    import jax
    from jax.sharding import Mesh, PartitionSpec
    from jax.experimental.shard_map import shard_map
    from concourse.bass2jax import (
        _bass_exec_p,
        install_neuronx_cc_hook,
        partition_id_tensor,
    )

    nc = build_bass()
    install_neuronx_cc_hook()

    partition_name = nc.partition_id_tensor.name if nc.partition_id_tensor else None

    in_names, out_names, out_avals = [], [], []
    for alloc in nc.m.functions[0].allocations:
        if not isinstance(alloc, mybir.MemoryLocationSet):
            continue
        name = alloc.memorylocations[0].name
        if alloc.kind == "ExternalInput":
            if name != partition_name:
                in_names.append(name)
        elif alloc.kind == "ExternalOutput":
            out_names.append(name)
            out_avals.append(
                jax.core.ShapedArray(
                    tuple(alloc.tensor_shape), mybir.dt.np(alloc.dtype)
                )
            )
    n_params = len(in_names)
    all_in_names = list(in_names) + list(out_names)
    if partition_name is not None:
        all_in_names.append(partition_name)

    def _body(*args):
        operands = list(args)
        if partition_name is not None:
            operands.append(partition_id_tensor())
        outs = _bass_exec_p.bind(
            *operands,
            out_avals=tuple(out_avals),
            in_names=tuple(all_in_names),
            out_names=tuple(out_names),
            lowering_input_output_aliases=(),
            sim_require_finite=True,
            sim_require_nnan=True,
            nc=nc,
        )
        return tuple(outs)

    devices = jax.devices()[:N_CORES]
    mesh = Mesh(np.asarray(devices), ("core",))
    n_outs = len(out_names)
    jitted = jax.jit(
        shard_map(
            _body,
            mesh=mesh,
            in_specs=(PartitionSpec("core"),) * (n_params + n_outs),
            out_specs=(PartitionSpec("core"),) * n_outs,
            check_rep=False,
        ),
        keep_unused=True,
    )

    def run(in_maps):
        concat_in = [
            np.concatenate(
                [np.asarray(in_maps[c][nm]) for c in range(N_CORES)], axis=0
            )
            for nm in in_names
        ]
        concat_zeros = [
            np.zeros((N_CORES * a.shape[0], *a.shape[1:]), a.dtype)
            for a in out_avals
        ]
        outs = jitted(*concat_in, *concat_zeros)
        return [
            {
                nm: np.asarray(outs[i]).reshape(N_CORES, *out_avals[i].shape)[c]
                for i, nm in enumerate(out_names)
            }
            for c in range(N_CORES)
        ]

    return run


def kernel(x, W1, b1, W2, b2, phases):
    """Full-input entry point. `phases` is unused: the reference's phase
    assignment is the static contiguous partition of the sequence."""
    global _RUNNER
    x = np.asarray(x, dtype=np.float32)
    W1 = np.asarray(W1, dtype=np.float32)
    b1 = np.asarray(b1, dtype=np.float32)
    W2 = np.asarray(W2, dtype=np.float32)
    b2 = np.asarray(b2, dtype=np.float32)

    if _RUNNER is None:
        _RUNNER = _make_runner()
    in_maps = _shard_inputs(x, W1, b1, W2)
    try:
        results = _RUNNER(in_maps)
    except Exception:
        # transient NRT device errors have been observed; retry once
        results = _RUNNER(in_maps)
    return _unshard_outputs(results, b2)


if __name__ == "__main__":
    rng = np.random.default_rng(0)
    x = rng.standard_normal((B, S, D), dtype=np.float32)
    W1 = (rng.random((P, D, F), dtype=np.float32) - 0.5) / np.sqrt(D)
    b1 = (rng.random((P, F), dtype=np.float32) - 0.5) / np.sqrt(D)
    W2 = (rng.random((P, F, D), dtype=np.float32) - 0.5) / np.sqrt(F)
    b2 = (rng.random((P, D), dtype=np.float32) - 0.5) / np.sqrt(F)
    phases = rng.integers(0, P, size=(B, S)).astype(np.int32)

    y = kernel(x, W1, b1, W2, b2, phases)

    xs = x.reshape(B, P, S // P, D)
    h = np.maximum(np.einsum("bpsd,pdf->bpsf", xs, W1) + b1[None, :, None, :], 0.0)
    yref = (np.einsum("bpsf,pfd->bpsd", h, W2) + b2[None, :, None, :]).reshape(B, S, D)
    err = np.linalg.norm(y - yref) / np.linalg.norm(yref)
    print("rel err:", err)

